# revision 24
# baseline (speedup 1.0000x reference)
"""Trainium2 Bass kernel for nn_Graph_CNN_Feat_Mesh (Chebyshev GNN decoder).

Strategy (per-core, data-parallel over batch B=256 -> 32/core):
  - All spmms are dense matmuls on the tensor engine (PE) in bf16.
    For K=3 Chebyshev conv:  y = A(x) + L @ B(x) + (2 L^2) @ C(x)
    with A = W0-W2, B = W1, C = W2 applied per-vertex in feature space.
    For up4-preceded layers, replication is folded into the host-side
    matrices:  y = A(x_up) + (L U) @ B(x320) + (2 L^2 U) @ C(x320),
    so both spmms contract over the small pre-upsample vertex space.
  - Layers c0-c2 run the spmm TRANSPOSED (lhsT = feature tiles, rhs = L
    tiles), emitting the next layer's packed F-layout directly: no
    back-transposes.  The A-term accumulates into the same PSUM with a
    stride-0 broadcast rhs for the up4 replication.
  - BatchNorm (training mode, global batch stats) is exact: per-core
    partial sums are AllGather'd across the 8 cores (cheaper than
    AllReduce) and summed locally; scale/shift+relu is applied in column
    chunks feeding the next layer's matmuls incrementally.
  - FC head runs in bf16 with fp32 PSUM accumulation; weight DMAs are
    issued in consumption order and big late-use matrices (L2, 2*L2^2)
    alias the FC weight SBUF space (chunked so the tiny BN collective
    DMAs never queue behind a long transfer).
"""

import numpy as np

B = 256
NCORES = 8
BL = B // NCORES  # 32
EPS = 1e-5

_CACHE = {}


def _split_W(W):
    W = np.asarray(W, np.float32)
    return W[:, 0::3], W[:, 1::3], W[:, 2::3]


def _dense_L(rows, cols, vals, V):
    L = np.zeros((V, V), np.float32)
    np.add.at(L, (np.asarray(rows), np.asarray(cols)), np.asarray(vals, np.float32))
    return L


def _pad_rows(a, m):
    if a.shape[0] % m == 0:
        return a
    p = m - a.shape[0] % m
    return np.concatenate([a, np.zeros((p,) + a.shape[1:], a.dtype)], 0)


def _stiles(a):
    """[U, V] -> [128, nS*V] with s-tiles of 128 source rows side by side."""
    a = _pad_rows(np.ascontiguousarray(a), 128)
    nS = a.shape[0] // 128
    return np.concatenate([a[s * 128:(s + 1) * 128, :] for s in range(nS)], axis=1)


def _wbd(M, G, Fin, Fout):
    """Block-diagonal weight [128, G*Fout]; block j holds M.T ([Fin, Fout])."""
    out = np.zeros((128, G * Fout), np.float32)
    for j in range(G):
        out[j * Fin:(j + 1) * Fin, j * Fout:(j + 1) * Fout] = M.T
    return out


class _LCfg:
    def __init__(self, name, Vin, Vsp, V, Fin, Fout, up4):
        self.name = name
        self.Vin = Vin            # per-g input column span of XF
        self.Vsp = Vsp            # source vertex space of B/C linears
        self.V = V                # output vertex count
        self.Fin = Fin
        self.Fout = Fout
        self.G = 128 // Fin       # input batch packs
        self.nG = BL // self.G
        self.GF = self.G * Fout
        self.Gp = 128 // Fout     # output batch packs
        self.BF = BL * Fout
        self.nGp = self.BF // 128  # output 128-col blocks
        self.nS = (Vsp + 127) // 128
        self.up4 = up4

    def sps(self, s):
        return min(128, self.Vsp - s * 128)


CFG = [
    _LCfg("c0", 80, 80, 320, 64, 64, True),
    _LCfg("c1", 320, 320, 320, 64, 32, False),
    _LCfg("c2", 320, 320, 1280, 32, 32, True),
]
# c3 (V-layout output layer): Fin=32, Fout=3, G=4, V=Vsp=1280


def _build_host(inputs):
    import ml_dtypes
    bf = ml_dtypes.bfloat16
    f32 = np.float32
    d = {}

    # ---- FC head ----
    xT = np.ascontiguousarray(np.asarray(inputs["x"], f32).T)  # [2048, 256]
    d["xTp_full"] = xT  # sliced + packed per core in kernel()
    fc1wT = np.ascontiguousarray(np.asarray(inputs["fc1_w"], f32).T)  # [2048, 512]
    d["fc1w"] = np.ascontiguousarray(
        fc1wT.reshape(16, 128, 512).transpose(1, 0, 2).reshape(128, 16 * 512)
    ).astype(bf)
    fc2wT = np.ascontiguousarray(np.asarray(inputs["fc2_w"], f32).T)  # [512, 5120]
    f2 = fc2wT.reshape(4, 128, 5120)
    for mc in range(4):
        d[f"fc2w{mc}"] = np.ascontiguousarray(
            f2[:, :, mc * 1280:(mc + 1) * 1280].transpose(1, 0, 2).reshape(128, 4 * 1280)
        ).astype(bf)
    smalls = np.zeros((128, 100), f32)
    for j in range(2):
        smalls[j * 64:(j + 1) * 64, 0:64] += np.eye(64, dtype=f32)
    for j in range(4):
        smalls[j * 32:(j + 1) * 32, 64:96] += np.eye(32, dtype=f32)
    smalls[:, 96:100] = np.asarray(inputs["fc1_b"], f32).reshape(4, 128).T
    d["smalls"] = smalls

    # ---- L matrices ----
    L1 = _dense_L(inputs["L1_rows"], inputs["L1_cols"], inputs["L1_vals"], 320)
    L2 = _dense_L(inputs["L2_rows"], inputs["L2_cols"], inputs["L2_vals"], 1280)
    U1 = np.repeat(np.eye(80, dtype=f32), 4, axis=0)    # [320, 80]
    U2 = np.repeat(np.eye(320, dtype=f32), 4, axis=0)   # [1280, 320]
    LU0 = (L1 @ U1).T                                   # [80, 320]
    LLU0 = 2.0 * (L1 @ (L1 @ U1)).T
    d["LU0p"] = _pad_rows(np.concatenate([LU0, LLU0], axis=1), 128).astype(bf)
    # [L ; 2L^2] stacked vertically -> 5 full 128-row K-tiles
    d["LT1p"] = _stiles(np.concatenate(
        [L1.T, 2.0 * (L1 @ L1).T], axis=0)).astype(bf)      # [128, 5*320]
    d["LU2p"] = _stiles(np.concatenate(
        [(L2 @ U2).T, 2.0 * (L2 @ (L2 @ U2)).T], axis=0)).astype(bf)  # [128, 5*1280]
    d["LT2"] = _stiles(L2.T).astype(bf)                 # [128, 10*1280]
    d["LL2"] = _stiles(2.0 * (L2 @ L2).T).astype(bf)

    # ---- Chebyshev linear weight blocks ----
    blks = []
    offs = {}

    def add(nm, arr):
        offs[nm] = sum(b.shape[1] for b in blks)
        blks.append(arr)

    for li, (cfg, wn) in enumerate(zip(CFG, ["cl0_w", "cl1_w", "cl2_w"])):
        W0, W1, W2 = _split_W(inputs[wn])
        A = W0 - W2
        add(f"B{li}", _wbd(W1, cfg.G, cfg.Fin, cfg.Fout))
        add(f"C{li}", _wbd(W2, cfg.G, cfg.Fin, cfg.Fout))
        if cfg.name == "c1":
            for dl in range(2):
                M = np.zeros((128, 128), f32)
                for j in range(2):
                    M[j * 64:(j + 1) * 64,
                      (2 * dl + j) * 32:(2 * dl + j + 1) * 32] = A.T
                add(f"A1_{dl}", M)
        else:
            add(f"A{li}", _wbd(A, cfg.G, cfg.Fin, cfg.Fout))
    W0, W1, W2 = _split_W(inputs["cl3_w"])
    add("B3", _wbd(W1, 4, 32, 3))
    add("C3", _wbd(W2, 4, 32, 3))
    add("A3", _wbd(W0 - W2, 4, 32, 3))
    d["wblk"] = np.concatenate(blks, axis=1).astype(bf)
    d["_woffs"] = offs  # not uploaded

    for i, (g, b) in enumerate([("bn0_g", "bn0_b"), ("bn1_g", "bn1_b"),
                                ("bn2_g", "bn2_b")]):
        gb = np.concatenate([np.asarray(inputs[g], f32),
                             np.asarray(inputs[b], f32)])
        d[f"gb{i}"] = np.ascontiguousarray(gb[None, :])  # [1, 2F]
    b3 = np.asarray(inputs["cl3_b"], f32)
    d["b3r"] = np.ascontiguousarray(np.tile(b3, 160)[None, :])  # [1, 480]
    return d


def _build_nc(woffs):
    import sys
    for p in ("/opt/trn_rl_repo", "/opt/trn_rl_repo/concourse"):
        if p not in sys.path:
            sys.path.insert(0, p)
    import concourse.bass as bass  # noqa
    import concourse.mybir as mybir
    import concourse.tile as tile
    from concourse import bacc
    from concourse.masks import make_identity

    f32 = mybir.dt.float32
    bf16 = mybir.dt.bfloat16
    AF = mybir.ActivationFunctionType
    ALU = mybir.AluOpType

    nc = bacc.Bacc(None, target_bir_lowering=False)

    xTp = nc.dram_tensor("xTp", [128, 16 * BL], bf16, kind="ExternalInput")
    smalls_d = nc.dram_tensor("smalls", [128, 100], f32, kind="ExternalInput")
    fc1w_d = nc.dram_tensor("fc1w", [128, 16 * 512], bf16, kind="ExternalInput")
    fc2w_d = [nc.dram_tensor(f"fc2w{mc}", [128, 4 * 1280], bf16,
                             kind="ExternalInput") for mc in range(4)]
    wblk_d = nc.dram_tensor("wblk", [128, 1188], bf16, kind="ExternalInput")
    LU0p_d = nc.dram_tensor("LU0p", [128, 640], bf16, kind="ExternalInput")
    LT1p_d = nc.dram_tensor("LT1p", [128, 1600], bf16, kind="ExternalInput")
    LU2p_d = nc.dram_tensor("LU2p", [128, 6400], bf16, kind="ExternalInput")
    LT2_d = nc.dram_tensor("LT2", [128, 12800], bf16, kind="ExternalInput")
    LL2_d = nc.dram_tensor("LL2", [128, 12800], bf16, kind="ExternalInput")
    gbs_d = [nc.dram_tensor(f"gb{i}", [1, 2 * F], f32, kind="ExternalInput")
             for i, F in enumerate([64, 32, 32])]
    b3r_d = nc.dram_tensor("b3r", [1, 480], f32, kind="ExternalInput")
    ydram = nc.dram_tensor("y", [128, 960], f32, kind="ExternalOutput")

    with tile.TileContext(nc) as tc:
        with (
            tc.tile_pool(name="wpool", bufs=1) as wpool,
            tc.tile_pool(name="actp", bufs=1) as actp,
            tc.tile_pool(name="misc", bufs=1) as miscp,
            tc.tile_pool(name="pslin", bufs=2, space="PSUM") as pslin,
            tc.tile_pool(name="psW", bufs=3, space="PSUM") as psW,
            tc.tile_pool(name="dram", bufs=1, space="DRAM") as dramp,
        ):
            # ================= SBUF tiles =================
            W1 = wpool.tile([128, 20480], bf16, tag="W1")      # fc2w
            W2 = wpool.tile([128, 8192], bf16, tag="W2")       # fc1w
            LU2p = wpool.tile([128, 6400], bf16, tag="LU2p2")
            LT2 = wpool.tile([128, 12800], bf16, tag="LT2")
            LL2 = wpool.tile([128, 12800], bf16, tag="LL2")
            wblk = wpool.tile([128, 1188], bf16, tag="wblk")
            LU0p = wpool.tile([128, 640], bf16, tag="LU0p")
            LT1p = wpool.tile([128, 1600], bf16, tag="LT1p")
            smalls = wpool.tile([128, 100], f32, tag="smalls")
            xT = wpool.tile([128, 16 * BL], bf16, tag="xT")
            gb_sb = [wpool.tile([1, 2 * F], f32, tag=f"gb{i}",
                                name=f"gb{i}")
                     for i, F in enumerate([64, 32, 32])]
            b3r = wpool.tile([1, 480], f32, tag="b3r")

            def WB(nm, w):
                return wblk[:, woffs[nm]:woffs[nm] + w]

            # ---- DMA issue order == consumption order (SP queue) ----
            nc.sync.dma_start(xT[:], xTp[:])
            nc.sync.dma_start(smalls[:], smalls_d[:])
            for k in range(4):
                nc.sync.dma_start(W2[:, k * 2048:(k + 1) * 2048],
                                  fc1w_d[:, k * 2048:(k + 1) * 2048])
            for mc in range(4):
                nc.sync.dma_start(W1[:, mc * 5120:(mc + 1) * 5120], fc2w_d[mc][:])
            nc.sync.dma_start(wblk[:], wblk_d[:])
            nc.sync.dma_start(LU0p[:], LU0p_d[:])
            nc.sync.dma_start(LT1p[:], LT1p_d[:])
            for i in range(3):
                nc.sync.dma_start(gb_sb[i][:], gbs_d[i][:])
            nc.sync.dma_start(b3r[:], b3r_d[:])
            # Late-use loads are emitted at compute milestones via late_load()
            # (a dummy gate write gives each chunk DMA a data dependency, so
            # the transfers never sit in the DMA_ENGINES queue ahead of the
            # tiny BN-collective DMAs).

            def late_load(dst, dsrc, nchunks, csz, dep):
                for k in range(nchunks):
                    nc.vector.tensor_copy(dst[0:1, k * csz:k * csz + 1], dep)
                    nc.sync.dma_start(dst[:, k * csz:(k + 1) * csz],
                                      dsrc[:, k * csz:(k + 1) * csz])

            # ---- constants / act-table warm ----
            eps_t = miscp.tile([1, 1], f32, tag="eps")
            nc.gpsimd.memset(eps_t[:], EPS)
            ones8 = miscp.tile([8, 1], f32, tag="ones8")
            nc.gpsimd.memset(ones8[:], 1.0)
            onesr = miscp.tile([1, 128], bf16, tag="onesr")
            nc.gpsimd.memset(onesr[:], 1.0)
            b3rb = miscp.tile([1, 480], bf16, tag="b3rb")
            nc.vector.tensor_copy(b3rb[:], b3r[:])
            ident_f = miscp.tile([128, 128], f32, tag="identf")
            make_identity(nc, ident_f[:])
            warm = miscp.tile([1, 4], f32, tag="warm")
            nc.gpsimd.memset(warm[:], 1.0)
            nc.scalar.activation(warm[:, 0:1], warm[:, 0:1], AF.Relu)
            nc.scalar.activation(warm[:, 1:2], warm[:, 1:2], AF.Copy)
            nc.scalar.activation(warm[:, 2:3], warm[:, 2:3], AF.Sqrt)

            # round-robin copy engines for PSUM->SBUF evacuation
            cp_state = [0]

            def cp(dst, src, eng=None):
                e = cp_state[0] % 2 if eng is None else eng
                cp_state[0] += 1
                if e == 0:
                    nc.scalar.activation(dst, src, AF.Copy)
                else:
                    nc.vector.tensor_copy(dst, src)

            # ================= FC head (bf16) =================
            # kt-outer so PE chases the chunked fc1w DMA; each mt group gets
            # its own PSUM bank (one open accumulation group per bank)
            h1T = miscp.tile([128, 4 * BL], bf16, tag="h1T")
            ps1a = psW.tile([128, 1024], f32, tag="big", name="ps1a")
            ps1b = psW.tile([128, 1024], f32, tag="big", name="ps1b")
            mtsl = [(ps1a, 0), (ps1a, 512), (ps1b, 0), (ps1b, 512)]
            for kt in range(16):
                for mt in range(4):
                    pt_, c0_ = mtsl[mt]
                    nc.tensor.matmul(
                        pt_[:, c0_:c0_ + BL],
                        W2[:, kt * 512 + mt * 128: kt * 512 + (mt + 1) * 128],
                        xT[:, kt * BL:(kt + 1) * BL],
                        start=(kt == 0), stop=(kt == 15),
                        skip_group_check=True)
            for mt in range(4):
                pt_, c0_ = mtsl[mt]
                nc.scalar.activation(
                    h1T[:, mt * BL:(mt + 1) * BL], pt_[:, c0_:c0_ + BL],
                    AF.Relu, bias=smalls[:, 96 + mt:97 + mt])

            XF0 = actp.tile([128, 16 * 80], bf16, tag="XF0")
            for mc in range(4):
                ps2 = psW.tile([128, 1024], f32, tag="big")
                for mi in range(10):
                    for kt in range(4):
                        nc.tensor.matmul(
                            ps2[:, mi * BL:(mi + 1) * BL],
                            W1[:, mc * 5120 + kt * 1280 + mi * 128:
                               mc * 5120 + kt * 1280 + (mi + 1) * 128],
                            h1T[:, kt * BL:(kt + 1) * BL],
                            start=(kt == 0), stop=(kt == 3),
                            skip_group_check=True)
                # psum [(v0%2)*64+f, b] -> XF0[(b%2)*64+f, (b//2)*80 + v0]
                src4 = ps2[:, 0:320].rearrange("p (i g j) -> p i g j", g=16, j=2)
                dst4 = XF0[:].rearrange("p (g u q) -> p g u q", u=40, q=2)
                for p0 in range(2):
                    for j in range(2):
                        nc.scalar.activation(
                            dst4[j * 64:(j + 1) * 64, :,
                                 mc * 10:(mc + 1) * 10, p0]
                            .rearrange("p g i -> p i g"),
                            src4[p0 * 64:(p0 + 1) * 64, :, :, j],
                            AF.Copy)

            # ================= cheby layers c0-c2 (F-layout) =================
            XF_cur = XF0

            for li, cfg in enumerate(CFG):
                V, Vin, F = cfg.V, cfg.Vin, cfg.Fout
                BF, nS, nGp = cfg.BF, cfg.nS, cfg.nGp
                # L-matrix rhs tiles: [128, nS*V (L-part) | nS*V (2L^2-part)]
                if cfg.name == "c0":
                    Lr, lw = LU0p, 320
                elif cfg.name == "c1":
                    Lr, lw = LT1p, 320
                else:
                    Lr, lw = LU2p, 1280

                # ---- B/C linears into source-vertex space ----
                XB = actp.tile([128, 5120], bf16, tag="XB",
                               name="XB")
                XC = actp.tile([128, 2048], bf16, tag="XC",
                               name="XC")
                gpack = max(1, 512 // cfg.GF)
                for s in range(nS):
                    ssz = cfg.sps(s)
                    for which, wnm in ((0, f"B{li}"), (1, f"C{li}")):
                        Wt = WB(wnm, cfg.GF)
                        for g0 in range(0, cfg.nG, gpack):
                            gn = min(gpack, cfg.nG - g0)
                            pc = pslin.tile([128, 512], f32, tag="lin")
                            for gi in range(gn):
                                g = g0 + gi
                                nc.tensor.matmul(
                                    pc[:ssz, gi * cfg.GF:(gi + 1) * cfg.GF],
                                    XF_cur[:, g * Vin + s * 128:
                                           g * Vin + s * 128 + ssz],
                                    Wt, start=True, stop=True,
                                    skip_group_check=True)
                            c0_, c1_ = g0 * cfg.GF, (g0 + gn) * cfg.GF
                            cw = c1_ - c0_
                            if li == 0:
                                dst = XB if which == 0 else XC
                                cp(dst[:ssz, s * BF + c0_:s * BF + c1_],
                                   pc[:ssz, :cw])
                            elif which == 0:
                                # B rows -> stacked tiles s (same partitions)
                                cp(XB[:ssz, s * BF + c0_:s * BF + c1_],
                                   pc[:ssz, :cw])
                            else:
                                # C rows land at stack offset Vsp=320: tile
                                # 2+s parts 64:, then tile 3+s parts :ssz-64
                                fh = min(64, ssz)
                                cp(XB[64:64 + fh,
                                      (2 + s) * BF + c0_:(2 + s) * BF + c1_],
                                   pc[0:fh, :cw])
                                if ssz > 64:
                                    cp(XB[0:ssz - 64,
                                          (3 + s) * BF + c0_:(3 + s) * BF + c1_],
                                       pc[64:ssz, :cw])

                if li == 0:
                    # pace LU2p load: gated on c0's first B-linear output so
                    # its transfers neither delay fc2w nor collide with the
                    # BN0 collective's DMA window
                    late_load(LU2p, LU2p_d, 4, 1600, XB[0:1, 0:1])

                # ---- transposed spmm + copies + 512-wide stats windows ----
                XFn = actp.tile([128, nGp * V], bf16, tag=f"XFn{li}")
                FD = nGp * V
                nch = FD // 512
                bnst = miscp.tile([128, nch * 6], f32, tag=f"bnst{li}")
                stat_done = [0, 0]  # cols copied, windows emitted

                def emit_stats(done, XFn=XFn, bnst=bnst, sd=stat_done, nch=nch):
                    sd[0] = done
                    while sd[1] < nch and (sd[1] + 1) * 512 <= sd[0]:
                        ci = sd[1]
                        nc.vector.bn_stats(
                            bnst[:, ci * 6:(ci + 1) * 6],
                            XFn[:, ci * 512:(ci + 1) * 512])
                        sd[1] += 1

                def spmm_group(ps, pbase, gp, w0, wcw):
                    """Accumulate output block (gp, w0:w0+wcw) into ps cols
                    pbase:pbase+wcw (wcw <= 512)."""
                    if li == 0:
                        for half, XS in ((0, XB), (1, XC)):
                            nc.tensor.matmul(
                                ps[:, pbase:pbase + wcw],
                                XS[:80, gp * 128:(gp + 1) * 128],
                                Lr[:80, half * lw + w0:half * lw + w0 + wcw],
                                start=(half == 0), stop=False,
                                skip_group_check=True)
                    else:
                        for st in range(5):
                            nc.tensor.matmul(
                                ps[:, pbase:pbase + wcw],
                                XB[:, st * BF + gp * 128:
                                   st * BF + (gp + 1) * 128],
                                Lr[:, st * lw + w0:st * lw + w0 + wcw],
                                start=(st == 0), stop=False,
                                skip_group_check=True)
                    if cfg.name == "c1":
                        for dl in range(2):
                            nc.tensor.matmul(
                                ps[:, pbase:pbase + wcw],
                                WB(f"A1_{dl}", 128),
                                XF_cur[:, (2 * gp + dl) * Vin + w0:
                                       (2 * gp + dl) * Vin + w0 + wcw],
                                start=False, stop=(dl == 1),
                                skip_group_check=True)
                    else:
                        rhs = XF_cur[:, gp * Vin + w0 // 4:
                                     gp * Vin + w0 // 4 + wcw // 4]
                        rhs = rhs.broadcast_to([128, wcw // 4, 4])
                        nc.tensor.matmul(
                            ps[:, pbase:pbase + wcw], WB(f"A{li}", 128), rhs,
                            start=False, stop=True, skip_group_check=True)

                if V <= 512:
                    # pack 2 gp-blocks per PSUM tile (bank-aligned at 512)
                    for gpp in range(0, nGp, 2):
                        ps = psW.tile([128, 1024], f32, tag="big")
                        for gi in range(2):
                            spmm_group(ps, gi * 512, gpp + gi, 0, V)
                        cp(XFn[:, gpp * V:(gpp + 2) * V]
                           .rearrange("p (u w) -> p u w", w=V),
                           ps[:].rearrange("p (u w) -> p u w", w=512)[:, :, :V],
                           eng=0 if gpp + 2 >= nGp else None)
                        emit_stats((gpp + 2) * V)
                else:
                    # c2: V=1280 per gp -> tiles of 1024 + 256
                    for gp in range(nGp):
                        for w0 in (0, 1024):
                            wcw = min(1024, V - w0)
                            ps = psW.tile([128, 1024], f32, tag="big")
                            for nk in range(0, wcw, 512):
                                sub = min(512, wcw - nk)
                                spmm_group(ps, nk, gp, w0 + nk, sub)
                            cp(XFn[:, gp * V + w0: gp * V + w0 + wcw],
                               ps[:, :wcw],
                               eng=0 if gp == nGp - 1 else None)
                            emit_stats(gp * V + w0 + wcw)

                # ---- BN: partial sums -> AllGather -> scale/shift ----
                n_g = float(B * V)
                aggr = miscp.tile([128, 2], f32, tag="aggr")
                nc.vector.bn_aggr(
                    aggr[:], bnst[:].rearrange("p (c s) -> p c s", s=6))
                part = miscp.tile([128, 2], f32, tag="part")
                nc.vector.tensor_tensor(
                    out=part[:, 1:2], in0=aggr[:, 0:1], in1=aggr[:, 0:1],
                    op=ALU.mult)
                nc.vector.tensor_tensor(
                    out=part[:, 1:2], in0=part[:, 1:2], in1=aggr[:, 1:2],
                    op=ALU.add)
                fscl = float(FD) / n_g
                nc.vector.tensor_scalar_mul(part[:, 1:2], part[:, 1:2], fscl)
                nc.vector.tensor_scalar_mul(part[:, 0:1], aggr[:, 0:1], fscl)
                sel = smalls[:, 0:64] if F == 64 else smalls[:, 64:96]
                pst = pslin.tile([128, 512], f32, tag="lin")
                nc.tensor.matmul(pst[:1, :F], part[:, 0:1], sel,
                                 start=True, stop=True, skip_group_check=True)
                nc.tensor.matmul(pst[:1, F:2 * F], part[:, 1:2], sel,
                                 start=True, stop=True, skip_group_check=True)
                stats_l = miscp.tile([1, 2 * F], f32, tag="statl")
                nc.vector.tensor_copy(stats_l[:], pst[:1, :2 * F])
                bin_ = dramp.tile([1, 2 * F], f32, tag=f"arin{li}")
                bout = dramp.tile([NCORES, 2 * F], f32, tag=f"arout{li}")
                nc.sync.dma_start(bin_[:], stats_l[:])
                nc.gpsimd.collective_compute(
                    "AllGather", ALU.bypass,
                    replica_groups=[list(range(NCORES))],
                    ins=[bin_.opt()], outs=[bout.opt()])
                statg8 = miscp.tile([NCORES, 2 * F], f32, tag="statg8")
                nc.sync.dma_start(statg8[:], bout[:])
                psg = pslin.tile([128, 512], f32, tag="lin")
                nc.tensor.matmul(psg[:1, :2 * F], ones8[:], statg8[:],
                                 start=True, stop=True, skip_group_check=True)
                # s,t from global [mu | Ex^2] (psg)
                st = miscp.tile([1, 2 * F], f32, tag="st")
                tmp = miscp.tile([1, 2 * F], f32, tag="sttmp")
                mu2 = miscp.tile([1, F], f32, tag="mu2")
                nc.vector.tensor_copy(tmp[:, :2 * F], psg[:1, :2 * F])
                nc.vector.tensor_tensor(out=mu2[:], in0=tmp[:, 0:F],
                                        in1=tmp[:, 0:F], op=ALU.mult)
                nc.vector.tensor_tensor(out=tmp[:, F:2 * F],
                                        in0=tmp[:, F:2 * F],
                                        in1=mu2[:], op=ALU.subtract)
                nc.scalar.activation(tmp[:, F:2 * F], tmp[:, F:2 * F],
                                     AF.Sqrt, bias=eps_t[:])
                nc.vector.reciprocal(tmp[:, F:2 * F], tmp[:, F:2 * F])
                nc.vector.tensor_tensor(out=st[:, 0:F],
                                        in0=tmp[:, F:2 * F],
                                        in1=gb_sb[li][:, 0:F], op=ALU.mult)
                nc.vector.tensor_tensor(out=mu2[:], in0=tmp[:, 0:F],
                                        in1=st[:, 0:F], op=ALU.mult)
                nc.vector.tensor_tensor(out=st[:, F:2 * F],
                                        in0=gb_sb[li][:, F:2 * F],
                                        in1=mu2[:], op=ALU.subtract)
                pss = pslin.tile([128, 512], f32, tag="lin", name="pss")
                nc.tensor.transpose(pss[:2 * F, 0:1], st[:],
                                    ident_f[:1, :1])
                stc = miscp.tile([128, 2], f32, tag=f"stc{li}")
                for j in range(cfg.Gp):
                    nc.vector.tensor_copy(stc[j * F:(j + 1) * F, 0:1],
                                          pss[:F, 0:1])
                    nc.vector.tensor_copy(stc[j * F:(j + 1) * F, 1:2],
                                          pss[F:2 * F, 0:1])
                # chunked scale/shift + relu, split across Act/DVE/Pool
                # (contiguous g-aligned chunks: subtile deps track them)
                def relu_chunk(ap, eng):
                    if eng == 1:
                        nc.vector.tensor_scalar(ap, ap, stc[:, 0:1],
                                                stc[:, 1:2], ALU.mult, ALU.add)
                        nc.vector.tensor_scalar_max(ap, ap, 0.0)
                    elif eng == 2:
                        nc.gpsimd.tensor_scalar(ap, ap, stc[:, 0:1],
                                                stc[:, 1:2], ALU.mult, ALU.add)
                        nc.gpsimd.tensor_scalar_max(ap, ap, 0.0)
                    else:
                        nc.scalar.activation(ap, ap, AF.Relu,
                                             scale=stc[:, 0:1],
                                             bias=stc[:, 1:2])

                csz = FD // 8
                # engine shares ~ inverse throughput: Act 3, DVE 5
                sched = [0, 1, 1, 0, 1, 1, 0, 1]
                for rc in range(8):
                    relu_chunk(XFn[:, rc * csz:(rc + 1) * csz], sched[rc])
                if li == 0:
                    late_load(LT2, LT2_d, 8, 1600, XFn[0:1, 0:1])
                elif li == 1:
                    late_load(LL2, LL2_d, 8, 1600, XFn[0:1, 0:1])
                XF_cur = XFn

            # ================= c3 (V-layout) + output =================
            # XF_cur = XF2 [128 (j4,c32), 8*1280], G=4, nG=8.
            # Per-s pipeline: relu chunk s -> B/C linears of source tile s ->
            # contributions of s accumulated into 2 persistent PSUM tiles
            # (t 0-4 and t 5-9), so the spmm overlaps the relu chunks.
            XB3 = actp.tile([128, 5120], bf16, tag="XB",
                            name="XB3")[:, :960]
            XC3 = actp.tile([128, 2048], bf16, tag="XC",
                            name="XC3")[:, :960]
            # B/C linears per source tile s, then contributions of s into 5
            # concurrently-open accumulators (one PSUM bank each) so wave A
            # (t 0-4) overlaps the relu/XB3 production; wave B (t 5-9) runs
            # back-to-back on the finished XB3/XC3.
            ysb = miscp.tile([128, 960], f32, tag="ysb")
            pwa = psW.tile([128, 1024], f32, tag="big", name="pwa")
            pwb = psW.tile([128, 1024], f32, tag="big", name="pwb")
            pwc = pslin.tile([128, 512], f32, tag="lin", name="pwc")
            wave_ps = [(pwa, 0), (pwa, 512), (pwb, 0), (pwb, 512), (pwc, 0)]

            def c3_wave(tbase, with_lin):
                for s in range(10):
                    if with_lin:
                        pc = pslin.tile([128, 512], f32, tag="lin")
                        for dst, wnm, off in ((XB3, "B3", 0), (XC3, "C3", 96)):
                            Wt = WB(wnm, 12)
                            for g in range(8):
                                nc.tensor.matmul(
                                    pc[:, off + g * 12: off + (g + 1) * 12],
                                    XF_cur[:, g * 1280 + s * 128:
                                           g * 1280 + (s + 1) * 128],
                                    Wt, start=True, stop=True,
                                    skip_group_check=True)
                        cp(XB3[:, s * 96:(s + 1) * 96], pc[:, 0:96])
                        cp(XC3[:, s * 96:(s + 1) * 96], pc[:, 96:192])
                    for ti in range(5):
                        t = tbase + ti
                        pv, po = wave_ps[ti]
                        nc.tensor.matmul(
                            pv[:, po:po + 96],
                            LT2[:, s * 1280 + t * 128:s * 1280 + (t + 1) * 128],
                            XB3[:, s * 96:(s + 1) * 96],
                            start=(s == 0), stop=False, skip_group_check=True)
                        nc.tensor.matmul(
                            pv[:, po:po + 96],
                            LL2[:, s * 1280 + t * 128:s * 1280 + (t + 1) * 128],
                            XC3[:, s * 96:(s + 1) * 96],
                            start=False, stop=False, skip_group_check=True)
                for ti in range(5):
                    t = tbase + ti
                    pv, po = wave_ps[ti]
                    for g in range(8):
                        nc.tensor.matmul(
                            pv[:, po + g * 12:po + (g + 1) * 12],
                            XF_cur[:, g * 1280 + t * 128:
                                   g * 1280 + (t + 1) * 128],
                            WB("A3", 12),
                            start=False, stop=False, skip_group_check=True)
                    nc.tensor.matmul(
                        pv[:, po:po + 96], onesr[:, :128], b3rb[:, 0:96],
                        start=False, stop=True, skip_group_check=True)
                    cp(ysb[:, t * 96:(t + 1) * 96], pv[:, po:po + 96])

            c3_wave(0, True)
            c3_wave(5, False)
            nc.sync.dma_start(ydram[:, 0:480], ysb[:, 0:480])
            nc.sync.dma_start(ydram[:, 480:960], ysb[:, 480:960])

    nc.compile()
    return nc


def kernel(**inputs):
    import sys
    for p in ("/opt/trn_rl_repo", "/opt/trn_rl_repo/concourse"):
        if p not in sys.path:
            sys.path.insert(0, p)
    from concourse.bass_utils import run_bass_kernel_spmd
    import ml_dtypes

    host = _build_host(inputs)
    woffs = host.pop("_woffs")
    xT_full = host.pop("xTp_full")

    key = ("nc",)
    if key not in _CACHE:
        _CACHE[key] = _build_nc(woffs)
    nc = _CACHE[key]

    in_maps = []
    for c in range(NCORES):
        m = dict(host)
        xc = xT_full[:, c * BL:(c + 1) * BL]  # [2048, 32]
        m["xTp"] = np.ascontiguousarray(
            xc.reshape(16, 128, BL).transpose(1, 0, 2).reshape(128, 16 * BL)
        ).astype(ml_dtypes.bfloat16)
        in_maps.append(m)
    res = run_bass_kernel_spmd(nc, in_maps, core_ids=list(range(NCORES)))
    outs = []
    for c in range(NCORES):
        y = res.results[c]["y"].astype(np.float32)  # [128, 960]
        outs.append(y.reshape(128, 10, BL, 3).transpose(2, 1, 0, 3)
                    .reshape(BL, 1280, 3))
    return np.concatenate(outs, axis=0)


if __name__ == "__main__":
    import reference as R
    inp = R.setup_inputs()
    inp = {k: np.asarray(v) for k, v in inp.items()}
    act = kernel(**inp)
    exp = np.asarray(R.reference(**inp))
    err = np.linalg.norm(act - exp) / np.linalg.norm(exp)
    print("Relative error:", err)


# revision 25
# speedup vs baseline: 1.0274x; 1.0274x over previous
"""Trainium2 Bass kernel for nn_Graph_CNN_Feat_Mesh (Chebyshev GNN decoder).

Strategy (per-core, data-parallel over batch B=256 -> 32/core):
  - All spmms are dense matmuls on the tensor engine (PE) in bf16.
    For K=3 Chebyshev conv:  y = A(x) + L @ B(x) + (2 L^2) @ C(x)
    with A = W0-W2, B = W1, C = W2 applied per-vertex in feature space.
    For up4-preceded layers, replication is folded into the host-side
    matrices:  y = A(x_up) + (L U) @ B(x320) + (2 L^2 U) @ C(x320),
    so both spmms contract over the small pre-upsample vertex space.
  - Layers c0-c2 run the spmm TRANSPOSED (lhsT = feature tiles, rhs = L
    tiles), emitting the next layer's packed F-layout directly: no
    back-transposes.  The A-term accumulates into the same PSUM with a
    stride-0 broadcast rhs for the up4 replication.
  - BatchNorm (training mode, global batch stats) is exact: per-core
    partial sums are AllGather'd across the 8 cores (cheaper than
    AllReduce) and summed locally; scale/shift+relu is applied in column
    chunks feeding the next layer's matmuls incrementally.
  - FC head runs in bf16 with fp32 PSUM accumulation; weight DMAs are
    issued in consumption order and big late-use matrices (L2, 2*L2^2)
    alias the FC weight SBUF space (chunked so the tiny BN collective
    DMAs never queue behind a long transfer).
"""

import numpy as np

B = 256
NCORES = 8
BL = B // NCORES  # 32
EPS = 1e-5

_CACHE = {}


def _split_W(W):
    W = np.asarray(W, np.float32)
    return W[:, 0::3], W[:, 1::3], W[:, 2::3]


def _dense_L(rows, cols, vals, V):
    L = np.zeros((V, V), np.float32)
    np.add.at(L, (np.asarray(rows), np.asarray(cols)), np.asarray(vals, np.float32))
    return L


def _pad_rows(a, m):
    if a.shape[0] % m == 0:
        return a
    p = m - a.shape[0] % m
    return np.concatenate([a, np.zeros((p,) + a.shape[1:], a.dtype)], 0)


def _stiles(a):
    """[U, V] -> [128, nS*V] with s-tiles of 128 source rows side by side."""
    a = _pad_rows(np.ascontiguousarray(a), 128)
    nS = a.shape[0] // 128
    return np.concatenate([a[s * 128:(s + 1) * 128, :] for s in range(nS)], axis=1)


def _wbd(M, G, Fin, Fout):
    """Block-diagonal weight [128, G*Fout]; block j holds M.T ([Fin, Fout])."""
    out = np.zeros((128, G * Fout), np.float32)
    for j in range(G):
        out[j * Fin:(j + 1) * Fin, j * Fout:(j + 1) * Fout] = M.T
    return out


class _LCfg:
    def __init__(self, name, Vin, Vsp, V, Fin, Fout, up4):
        self.name = name
        self.Vin = Vin            # per-g input column span of XF
        self.Vsp = Vsp            # source vertex space of B/C linears
        self.V = V                # output vertex count
        self.Fin = Fin
        self.Fout = Fout
        self.G = 128 // Fin       # input batch packs
        self.nG = BL // self.G
        self.GF = self.G * Fout
        self.Gp = 128 // Fout     # output batch packs
        self.BF = BL * Fout
        self.nGp = self.BF // 128  # output 128-col blocks
        self.nS = (Vsp + 127) // 128
        self.up4 = up4

    def sps(self, s):
        return min(128, self.Vsp - s * 128)


CFG = [
    _LCfg("c0", 80, 80, 320, 64, 64, True),
    _LCfg("c1", 320, 320, 320, 64, 32, False),
    _LCfg("c2", 320, 320, 1280, 32, 32, True),
]
# c3 (V-layout output layer): Fin=32, Fout=3, G=4, V=Vsp=1280


def _build_host(inputs):
    import ml_dtypes
    bf = ml_dtypes.bfloat16
    f32 = np.float32
    d = {}

    # ---- FC head ----
    xT = np.ascontiguousarray(np.asarray(inputs["x"], f32).T)  # [2048, 256]
    d["xTp_full"] = xT  # sliced + packed per core in kernel()
    fc1wT = np.ascontiguousarray(np.asarray(inputs["fc1_w"], f32).T)  # [2048, 512]
    d["fc1w"] = np.ascontiguousarray(
        fc1wT.reshape(16, 128, 512).transpose(1, 0, 2).reshape(128, 16 * 512)
    ).astype(bf)
    fc2wT = np.ascontiguousarray(np.asarray(inputs["fc2_w"], f32).T)  # [512, 5120]
    f2 = fc2wT.reshape(4, 128, 5120)
    for mc in range(4):
        d[f"fc2w{mc}"] = np.ascontiguousarray(
            f2[:, :, mc * 1280:(mc + 1) * 1280].transpose(1, 0, 2).reshape(128, 4 * 1280)
        ).astype(bf)
    smalls = np.zeros((128, 100), f32)
    for j in range(2):
        smalls[j * 64:(j + 1) * 64, 0:64] += np.eye(64, dtype=f32)
    for j in range(4):
        smalls[j * 32:(j + 1) * 32, 64:96] += np.eye(32, dtype=f32)
    smalls[:, 96:100] = np.asarray(inputs["fc1_b"], f32).reshape(4, 128).T
    d["smalls"] = smalls

    # ---- L matrices ----
    L1 = _dense_L(inputs["L1_rows"], inputs["L1_cols"], inputs["L1_vals"], 320)
    L2 = _dense_L(inputs["L2_rows"], inputs["L2_cols"], inputs["L2_vals"], 1280)
    U1 = np.repeat(np.eye(80, dtype=f32), 4, axis=0)    # [320, 80]
    U2 = np.repeat(np.eye(320, dtype=f32), 4, axis=0)   # [1280, 320]
    LU0 = (L1 @ U1).T                                   # [80, 320]
    LLU0 = 2.0 * (L1 @ (L1 @ U1)).T
    d["LU0p"] = _pad_rows(np.concatenate([LU0, LLU0], axis=1), 128).astype(bf)
    # [L ; 2L^2] stacked vertically -> 5 full 128-row K-tiles
    d["LT1p"] = _stiles(np.concatenate(
        [L1.T, 2.0 * (L1 @ L1).T], axis=0)).astype(bf)      # [128, 5*320]
    d["LU2p"] = _stiles(np.concatenate(
        [(L2 @ U2).T, 2.0 * (L2 @ (L2 @ U2)).T], axis=0)).astype(bf)  # [128, 5*1280]
    d["LT2"] = _stiles(L2.T).astype(bf)                 # [128, 10*1280]
    d["LL2"] = _stiles(2.0 * (L2 @ L2).T).astype(bf)

    # ---- Chebyshev linear weight blocks ----
    blks = []
    offs = {}

    def add(nm, arr):
        offs[nm] = sum(b.shape[1] for b in blks)
        blks.append(arr)

    for li, (cfg, wn) in enumerate(zip(CFG, ["cl0_w", "cl1_w", "cl2_w"])):
        W0, W1, W2 = _split_W(inputs[wn])
        A = W0 - W2
        add(f"B{li}", _wbd(W1, cfg.G, cfg.Fin, cfg.Fout))
        add(f"C{li}", _wbd(W2, cfg.G, cfg.Fin, cfg.Fout))
        if cfg.name == "c1":
            for dl in range(2):
                M = np.zeros((128, 128), f32)
                for j in range(2):
                    M[j * 64:(j + 1) * 64,
                      (2 * dl + j) * 32:(2 * dl + j + 1) * 32] = A.T
                add(f"A1_{dl}", M)
        else:
            add(f"A{li}", _wbd(A, cfg.G, cfg.Fin, cfg.Fout))
    W0, W1, W2 = _split_W(inputs["cl3_w"])
    add("B3", _wbd(W1, 4, 32, 3))
    add("C3", _wbd(W2, 4, 32, 3))
    add("A3", _wbd(W0 - W2, 4, 32, 3))
    d["wblk"] = np.concatenate(blks, axis=1).astype(bf)
    d["_woffs"] = offs  # not uploaded

    for i, (g, b) in enumerate([("bn0_g", "bn0_b"), ("bn1_g", "bn1_b"),
                                ("bn2_g", "bn2_b")]):
        gb = np.concatenate([np.asarray(inputs[g], f32),
                             np.asarray(inputs[b], f32)])
        d[f"gb{i}"] = np.ascontiguousarray(gb[None, :])  # [1, 2F]
    b3 = np.asarray(inputs["cl3_b"], f32)
    d["b3r"] = np.ascontiguousarray(np.tile(b3, 160)[None, :])  # [1, 480]
    return d


def _build_nc(woffs):
    import sys
    for p in ("/opt/trn_rl_repo", "/opt/trn_rl_repo/concourse"):
        if p not in sys.path:
            sys.path.insert(0, p)
    import concourse.bass as bass  # noqa
    import concourse.mybir as mybir
    import concourse.tile as tile
    from concourse import bacc
    from concourse.masks import make_identity

    f32 = mybir.dt.float32
    bf16 = mybir.dt.bfloat16
    AF = mybir.ActivationFunctionType
    ALU = mybir.AluOpType

    nc = bacc.Bacc(None, target_bir_lowering=False)

    xTp = nc.dram_tensor("xTp", [128, 16 * BL], bf16, kind="ExternalInput")
    smalls_d = nc.dram_tensor("smalls", [128, 100], f32, kind="ExternalInput")
    fc1w_d = nc.dram_tensor("fc1w", [128, 16 * 512], bf16, kind="ExternalInput")
    fc2w_d = [nc.dram_tensor(f"fc2w{mc}", [128, 4 * 1280], bf16,
                             kind="ExternalInput") for mc in range(4)]
    wblk_d = nc.dram_tensor("wblk", [128, 1188], bf16, kind="ExternalInput")
    LU0p_d = nc.dram_tensor("LU0p", [128, 640], bf16, kind="ExternalInput")
    LT1p_d = nc.dram_tensor("LT1p", [128, 1600], bf16, kind="ExternalInput")
    LU2p_d = nc.dram_tensor("LU2p", [128, 6400], bf16, kind="ExternalInput")
    LT2_d = nc.dram_tensor("LT2", [128, 12800], bf16, kind="ExternalInput")
    LL2_d = nc.dram_tensor("LL2", [128, 12800], bf16, kind="ExternalInput")
    gbs_d = [nc.dram_tensor(f"gb{i}", [1, 2 * F], f32, kind="ExternalInput")
             for i, F in enumerate([64, 32, 32])]
    b3r_d = nc.dram_tensor("b3r", [1, 480], f32, kind="ExternalInput")
    ydram = nc.dram_tensor("y", [128, 960], f32, kind="ExternalOutput")

    with tile.TileContext(nc) as tc:
        with (
            tc.tile_pool(name="wpool", bufs=1) as wpool,
            tc.tile_pool(name="actp", bufs=1) as actp,
            tc.tile_pool(name="misc", bufs=1) as miscp,
            tc.tile_pool(name="pslin", bufs=2, space="PSUM") as pslin,
            tc.tile_pool(name="psW", bufs=3, space="PSUM") as psW,
            tc.tile_pool(name="dram", bufs=1, space="DRAM") as dramp,
        ):
            # ================= SBUF tiles =================
            W1 = wpool.tile([128, 20480], bf16, tag="W1")      # fc2w
            W2 = wpool.tile([128, 8192], bf16, tag="W2")       # fc1w
            LU2p = wpool.tile([128, 6400], bf16, tag="LU2p2")
            LT2 = wpool.tile([128, 12800], bf16, tag="LT2")
            LL2 = wpool.tile([128, 12800], bf16, tag="LL2")
            wblk = wpool.tile([128, 1188], bf16, tag="wblk")
            LU0p = wpool.tile([128, 640], bf16, tag="LU0p")
            LT1p = wpool.tile([128, 1600], bf16, tag="LT1p")
            smalls = wpool.tile([128, 100], f32, tag="smalls")
            xT = wpool.tile([128, 16 * BL], bf16, tag="xT")
            gb_sb = [wpool.tile([1, 2 * F], f32, tag=f"gb{i}",
                                name=f"gb{i}")
                     for i, F in enumerate([64, 32, 32])]
            b3r = wpool.tile([1, 480], f32, tag="b3r")

            def WB(nm, w):
                return wblk[:, woffs[nm]:woffs[nm] + w]

            # ---- DMA issue order == consumption order (SP queue) ----
            nc.sync.dma_start(xT[:], xTp[:])
            nc.sync.dma_start(smalls[:], smalls_d[:])
            for k in range(4):
                nc.sync.dma_start(W2[:, k * 2048:(k + 1) * 2048],
                                  fc1w_d[:, k * 2048:(k + 1) * 2048])
            for mc in range(4):
                nc.sync.dma_start(W1[:, mc * 5120:(mc + 1) * 5120], fc2w_d[mc][:])
            nc.sync.dma_start(wblk[:], wblk_d[:])
            nc.sync.dma_start(LU0p[:], LU0p_d[:])
            nc.sync.dma_start(LT1p[:], LT1p_d[:])
            for i in range(3):
                nc.sync.dma_start(gb_sb[i][:], gbs_d[i][:])
            nc.sync.dma_start(b3r[:], b3r_d[:])
            # Late-use loads are emitted at compute milestones via late_load()
            # (a dummy gate write gives each chunk DMA a data dependency, so
            # the transfers never sit in the DMA_ENGINES queue ahead of the
            # tiny BN-collective DMAs).

            def late_load(dst, dsrc, nchunks, csz, dep):
                for k in range(nchunks):
                    nc.vector.tensor_copy(dst[0:1, k * csz:k * csz + 1], dep)
                    nc.sync.dma_start(dst[:, k * csz:(k + 1) * csz],
                                      dsrc[:, k * csz:(k + 1) * csz])

            # ---- constants / act-table warm ----
            eps_t = miscp.tile([1, 1], f32, tag="eps")
            nc.gpsimd.memset(eps_t[:], EPS)
            ones8 = miscp.tile([8, 1], f32, tag="ones8")
            nc.gpsimd.memset(ones8[:], 1.0)
            onesr = miscp.tile([1, 128], bf16, tag="onesr")
            nc.gpsimd.memset(onesr[:], 1.0)
            b3rb = miscp.tile([1, 480], bf16, tag="b3rb")
            nc.vector.tensor_copy(b3rb[:], b3r[:])
            ident_f = miscp.tile([128, 128], f32, tag="identf")
            make_identity(nc, ident_f[:])
            warm = miscp.tile([1, 4], f32, tag="warm")
            nc.gpsimd.memset(warm[:], 1.0)
            nc.scalar.activation(warm[:, 0:1], warm[:, 0:1], AF.Relu)
            nc.scalar.activation(warm[:, 1:2], warm[:, 1:2], AF.Copy)
            nc.scalar.activation(warm[:, 2:3], warm[:, 2:3], AF.Sqrt)

            # round-robin copy engines for PSUM->SBUF evacuation
            cp_state = [0]

            def cp(dst, src, eng=None):
                e = cp_state[0] % 2 if eng is None else eng
                cp_state[0] += 1
                if e == 0:
                    nc.scalar.activation(dst, src, AF.Copy)
                else:
                    nc.vector.tensor_copy(dst, src)

            # ================= FC head (bf16) =================
            # kt-outer so PE chases the chunked fc1w DMA; each mt group gets
            # its own PSUM bank (one open accumulation group per bank)
            h1T = miscp.tile([128, 4 * BL], bf16, tag="h1T")
            ps1a = psW.tile([128, 1024], f32, tag="big", name="ps1a")
            ps1b = psW.tile([128, 1024], f32, tag="big", name="ps1b")
            mtsl = [(ps1a, 0), (ps1a, 512), (ps1b, 0), (ps1b, 512)]
            for kt in range(16):
                for mt in range(4):
                    pt_, c0_ = mtsl[mt]
                    nc.tensor.matmul(
                        pt_[:, c0_:c0_ + BL],
                        W2[:, kt * 512 + mt * 128: kt * 512 + (mt + 1) * 128],
                        xT[:, kt * BL:(kt + 1) * BL],
                        start=(kt == 0), stop=(kt == 15),
                        skip_group_check=True)
            for mt in range(4):
                pt_, c0_ = mtsl[mt]
                nc.scalar.activation(
                    h1T[:, mt * BL:(mt + 1) * BL], pt_[:, c0_:c0_ + BL],
                    AF.Relu, bias=smalls[:, 96 + mt:97 + mt])

            XF0 = actp.tile([128, 16 * 80], bf16, tag="XF0")
            for mc in range(4):
                ps2 = psW.tile([128, 1024], f32, tag="big")
                for mi in range(10):
                    for kt in range(4):
                        nc.tensor.matmul(
                            ps2[:, mi * BL:(mi + 1) * BL],
                            W1[:, mc * 5120 + kt * 1280 + mi * 128:
                               mc * 5120 + kt * 1280 + (mi + 1) * 128],
                            h1T[:, kt * BL:(kt + 1) * BL],
                            start=(kt == 0), stop=(kt == 3),
                            skip_group_check=True)
                # psum [(v0%2)*64+f, b] -> XF0[(b%2)*64+f, (b//2)*80 + v0]
                src4 = ps2[:, 0:320].rearrange("p (i g j) -> p i g j", g=16, j=2)
                dst4 = XF0[:].rearrange("p (g u q) -> p g u q", u=40, q=2)
                for p0 in range(2):
                    for j in range(2):
                        nc.scalar.activation(
                            dst4[j * 64:(j + 1) * 64, :,
                                 mc * 10:(mc + 1) * 10, p0]
                            .rearrange("p g i -> p i g"),
                            src4[p0 * 64:(p0 + 1) * 64, :, :, j],
                            AF.Copy)

            # ================= cheby layers c0-c2 (F-layout) =================
            XF_cur = XF0

            for li, cfg in enumerate(CFG):
                V, Vin, F = cfg.V, cfg.Vin, cfg.Fout
                BF, nS, nGp = cfg.BF, cfg.nS, cfg.nGp
                # L-matrix rhs tiles: [128, nS*V (L-part) | nS*V (2L^2-part)]
                if cfg.name == "c0":
                    Lr, lw = LU0p, 320
                elif cfg.name == "c1":
                    Lr, lw = LT1p, 320
                else:
                    Lr, lw = LU2p, 1280

                # ---- B/C linears into source-vertex space ----
                XB = actp.tile([128, 5120], bf16, tag="XB",
                               name="XB")
                XC = actp.tile([128, 2048], bf16, tag="XC",
                               name="XC")
                gpack = max(1, 512 // cfg.GF)
                for s in range(nS):
                    ssz = cfg.sps(s)
                    for which, wnm in ((0, f"B{li}"), (1, f"C{li}")):
                        Wt = WB(wnm, cfg.GF)
                        for g0 in range(0, cfg.nG, gpack):
                            gn = min(gpack, cfg.nG - g0)
                            pc = pslin.tile([128, 512], f32, tag="lin")
                            for gi in range(gn):
                                g = g0 + gi
                                nc.tensor.matmul(
                                    pc[:ssz, gi * cfg.GF:(gi + 1) * cfg.GF],
                                    XF_cur[:, g * Vin + s * 128:
                                           g * Vin + s * 128 + ssz],
                                    Wt, start=True, stop=True,
                                    skip_group_check=True)
                            c0_, c1_ = g0 * cfg.GF, (g0 + gn) * cfg.GF
                            cw = c1_ - c0_
                            if li == 0:
                                dst = XB if which == 0 else XC
                                cp(dst[:ssz, s * BF + c0_:s * BF + c1_],
                                   pc[:ssz, :cw])
                            elif which == 0:
                                # B rows -> stacked tiles s (same partitions)
                                cp(XB[:ssz, s * BF + c0_:s * BF + c1_],
                                   pc[:ssz, :cw])
                            else:
                                # C rows land at stack offset Vsp=320: tile
                                # 2+s parts 64:, then tile 3+s parts :ssz-64
                                fh = min(64, ssz)
                                cp(XB[64:64 + fh,
                                      (2 + s) * BF + c0_:(2 + s) * BF + c1_],
                                   pc[0:fh, :cw])
                                if ssz > 64:
                                    cp(XB[0:ssz - 64,
                                          (3 + s) * BF + c0_:(3 + s) * BF + c1_],
                                       pc[64:ssz, :cw])

                if li == 0:
                    # pace LU2p load: gated on c0's first B-linear output so
                    # its transfers neither delay fc2w nor collide with the
                    # BN0 collective's DMA window
                    late_load(LU2p, LU2p_d, 4, 1600, XB[0:1, 0:1])

                # ---- transposed spmm + copies + 512-wide stats windows ----
                XFn = actp.tile([128, nGp * V], bf16, tag=f"XFn{li}")
                FD = nGp * V
                nch = FD // 512
                bnst = miscp.tile([128, nch * 6], f32, tag=f"bnst{li}")
                stat_done = [0, 0]  # cols copied, windows emitted

                def emit_stats(done, XFn=XFn, bnst=bnst, sd=stat_done, nch=nch):
                    sd[0] = done
                    while sd[1] < nch and (sd[1] + 1) * 512 <= sd[0]:
                        ci = sd[1]
                        nc.vector.bn_stats(
                            bnst[:, ci * 6:(ci + 1) * 6],
                            XFn[:, ci * 512:(ci + 1) * 512])
                        sd[1] += 1

                def spmm_group(ps, pbase, gp, w0, wcw):
                    """Accumulate output block (gp, w0:w0+wcw) into ps cols
                    pbase:pbase+wcw (wcw <= 512)."""
                    if li == 0:
                        for half, XS in ((0, XB), (1, XC)):
                            nc.tensor.matmul(
                                ps[:, pbase:pbase + wcw],
                                XS[:80, gp * 128:(gp + 1) * 128],
                                Lr[:80, half * lw + w0:half * lw + w0 + wcw],
                                start=(half == 0), stop=False,
                                skip_group_check=True)
                    else:
                        for st in range(5):
                            nc.tensor.matmul(
                                ps[:, pbase:pbase + wcw],
                                XB[:, st * BF + gp * 128:
                                   st * BF + (gp + 1) * 128],
                                Lr[:, st * lw + w0:st * lw + w0 + wcw],
                                start=(st == 0), stop=False,
                                skip_group_check=True)
                    if cfg.name == "c1":
                        for dl in range(2):
                            nc.tensor.matmul(
                                ps[:, pbase:pbase + wcw],
                                WB(f"A1_{dl}", 128),
                                XF_cur[:, (2 * gp + dl) * Vin + w0:
                                       (2 * gp + dl) * Vin + w0 + wcw],
                                start=False, stop=(dl == 1),
                                skip_group_check=True)
                    else:
                        rhs = XF_cur[:, gp * Vin + w0 // 4:
                                     gp * Vin + w0 // 4 + wcw // 4]
                        rhs = rhs.broadcast_to([128, wcw // 4, 4])
                        nc.tensor.matmul(
                            ps[:, pbase:pbase + wcw], WB(f"A{li}", 128), rhs,
                            start=False, stop=True, skip_group_check=True)

                if V <= 512:
                    # pack 2 gp-blocks per PSUM tile (bank-aligned at 512)
                    for gpp in range(0, nGp, 2):
                        ps = psW.tile([128, 1024], f32, tag="big")
                        for gi in range(2):
                            spmm_group(ps, gi * 512, gpp + gi, 0, V)
                        cp(XFn[:, gpp * V:(gpp + 2) * V]
                           .rearrange("p (u w) -> p u w", w=V),
                           ps[:].rearrange("p (u w) -> p u w", w=512)[:, :, :V],
                           eng=0 if gpp + 2 >= nGp else None)
                        emit_stats((gpp + 2) * V)
                else:
                    # c2: V=1280 per gp -> tiles of 1024 + 256
                    for gp in range(nGp):
                        for w0 in (0, 1024):
                            wcw = min(1024, V - w0)
                            ps = psW.tile([128, 1024], f32, tag="big")
                            for nk in range(0, wcw, 512):
                                sub = min(512, wcw - nk)
                                spmm_group(ps, nk, gp, w0 + nk, sub)
                            cp(XFn[:, gp * V + w0: gp * V + w0 + wcw],
                               ps[:, :wcw],
                               eng=0 if gp == nGp - 1 else None)
                            emit_stats(gp * V + w0 + wcw)

                # ---- BN: partial sums -> AllGather -> scale/shift ----
                n_g = float(B * V)
                aggr = miscp.tile([128, 2], f32, tag="aggr")
                nc.vector.bn_aggr(
                    aggr[:], bnst[:].rearrange("p (c s) -> p c s", s=6))
                part = miscp.tile([128, 2], f32, tag="part")
                nc.vector.tensor_tensor(
                    out=part[:, 1:2], in0=aggr[:, 0:1], in1=aggr[:, 0:1],
                    op=ALU.mult)
                nc.vector.tensor_tensor(
                    out=part[:, 1:2], in0=part[:, 1:2], in1=aggr[:, 1:2],
                    op=ALU.add)
                fscl = float(FD) / n_g
                nc.vector.tensor_scalar_mul(part[:, 1:2], part[:, 1:2], fscl)
                nc.vector.tensor_scalar_mul(part[:, 0:1], aggr[:, 0:1], fscl)
                sel = smalls[:, 0:64] if F == 64 else smalls[:, 64:96]
                pst = pslin.tile([128, 512], f32, tag="lin")
                nc.tensor.matmul(pst[:1, :F], part[:, 0:1], sel,
                                 start=True, stop=True, skip_group_check=True)
                nc.tensor.matmul(pst[:1, F:2 * F], part[:, 1:2], sel,
                                 start=True, stop=True, skip_group_check=True)
                stats_l = miscp.tile([1, 2 * F], f32, tag="statl")
                nc.vector.tensor_copy(stats_l[:], pst[:1, :2 * F])
                bin_ = dramp.tile([1, 2 * F], f32, tag=f"arin{li}")
                bout = dramp.tile([NCORES, 2 * F], f32, tag=f"arout{li}")
                nc.sync.dma_start(bin_[:], stats_l[:])
                nc.gpsimd.collective_compute(
                    "AllGather", ALU.bypass,
                    replica_groups=[list(range(NCORES))],
                    ins=[bin_.opt()], outs=[bout.opt()])
                statg8 = miscp.tile([NCORES, 2 * F], f32, tag="statg8")
                nc.sync.dma_start(statg8[:], bout[:])
                psg = pslin.tile([128, 512], f32, tag="lin")
                nc.tensor.matmul(psg[:1, :2 * F], ones8[:], statg8[:],
                                 start=True, stop=True, skip_group_check=True)
                # s,t from global [mu | Ex^2] (psg)
                st = miscp.tile([1, 2 * F], f32, tag="st")
                tmp = miscp.tile([1, 2 * F], f32, tag="sttmp")
                mu2 = miscp.tile([1, F], f32, tag="mu2")
                nc.vector.tensor_copy(tmp[:, :2 * F], psg[:1, :2 * F])
                nc.vector.tensor_tensor(out=mu2[:], in0=tmp[:, 0:F],
                                        in1=tmp[:, 0:F], op=ALU.mult)
                nc.vector.tensor_tensor(out=tmp[:, F:2 * F],
                                        in0=tmp[:, F:2 * F],
                                        in1=mu2[:], op=ALU.subtract)
                nc.scalar.activation(tmp[:, F:2 * F], tmp[:, F:2 * F],
                                     AF.Sqrt, bias=eps_t[:])
                nc.vector.reciprocal(tmp[:, F:2 * F], tmp[:, F:2 * F])
                nc.vector.tensor_tensor(out=st[:, 0:F],
                                        in0=tmp[:, F:2 * F],
                                        in1=gb_sb[li][:, 0:F], op=ALU.mult)
                nc.vector.tensor_tensor(out=mu2[:], in0=tmp[:, 0:F],
                                        in1=st[:, 0:F], op=ALU.mult)
                nc.vector.tensor_tensor(out=st[:, F:2 * F],
                                        in0=gb_sb[li][:, F:2 * F],
                                        in1=mu2[:], op=ALU.subtract)
                pss = pslin.tile([128, 512], f32, tag="lin", name="pss")
                nc.tensor.transpose(pss[:2 * F, 0:1], st[:],
                                    ident_f[:1, :1])
                stc = miscp.tile([128, 2], f32, tag=f"stc{li}")
                for j in range(cfg.Gp):
                    nc.vector.tensor_copy(stc[j * F:(j + 1) * F, 0:1],
                                          pss[:F, 0:1])
                    nc.vector.tensor_copy(stc[j * F:(j + 1) * F, 1:2],
                                          pss[F:2 * F, 0:1])
                # chunked scale/shift + relu, split across Act/DVE/Pool
                # (contiguous g-aligned chunks: subtile deps track them)
                def relu_chunk(ap, eng):
                    if eng == 1:
                        nc.vector.tensor_scalar(ap, ap, stc[:, 0:1],
                                                stc[:, 1:2], ALU.mult, ALU.add)
                        nc.vector.tensor_scalar_max(ap, ap, 0.0)
                    elif eng == 2:
                        nc.gpsimd.tensor_scalar(ap, ap, stc[:, 0:1],
                                                stc[:, 1:2], ALU.mult, ALU.add)
                        nc.gpsimd.tensor_scalar_max(ap, ap, 0.0)
                    else:
                        nc.scalar.activation(ap, ap, AF.Relu,
                                             scale=stc[:, 0:1],
                                             bias=stc[:, 1:2])

                csz = FD // 8
                # engine shares: Act 4, DVE 4 (Pool's 2-op path straggles)
                sched = [0, 1, 0, 1, 1, 0, 1, 0]
                for rc in range(8):
                    relu_chunk(XFn[:, rc * csz:(rc + 1) * csz], sched[rc])
                if li == 0:
                    late_load(LT2, LT2_d, 8, 1600, XFn[0:1, 0:1])
                elif li == 1:
                    late_load(LL2, LL2_d, 8, 1600, XFn[0:1, 0:1])
                XF_cur = XFn

            # ================= c3 (V-layout) + output =================
            # XF_cur = XF2 [128 (j4,c32), 8*1280], G=4, nG=8.
            # Per-s pipeline: relu chunk s -> B/C linears of source tile s ->
            # contributions of s accumulated into 2 persistent PSUM tiles
            # (t 0-4 and t 5-9), so the spmm overlaps the relu chunks.
            XB3 = actp.tile([128, 5120], bf16, tag="XB",
                            name="XB3")[:, :960]
            XC3 = actp.tile([128, 2048], bf16, tag="XC",
                            name="XC3")[:, :960]
            for s in range(10):
                pc = pslin.tile([128, 512], f32, tag="lin")
                for dst, wnm, off in ((XB3, "B3", 0), (XC3, "C3", 96)):
                    Wt = WB(wnm, 12)
                    for g in range(8):
                        nc.tensor.matmul(
                            pc[:, off + g * 12: off + (g + 1) * 12],
                            XF_cur[:, g * 1280 + s * 128:
                                   g * 1280 + (s + 1) * 128],
                            Wt, start=True, stop=True, skip_group_check=True)
                cp(XB3[:, s * 96:(s + 1) * 96], pc[:, 0:96])
                cp(XC3[:, s * 96:(s + 1) * 96], pc[:, 96:192])
            ysb = miscp.tile([128, 960], f32, tag="ysb")
            for t in range(10):
                pv = psW.tile([128, 1024], f32, tag="big")
                for s in range(10):
                    nc.tensor.matmul(
                        pv[:, 0:96],
                        LT2[:, s * 1280 + t * 128:s * 1280 + (t + 1) * 128],
                        XB3[:, s * 96:(s + 1) * 96],
                        start=(s == 0), stop=False, skip_group_check=True)
                for s in range(10):
                    nc.tensor.matmul(
                        pv[:, 0:96],
                        LL2[:, s * 1280 + t * 128:s * 1280 + (t + 1) * 128],
                        XC3[:, s * 96:(s + 1) * 96],
                        start=False, stop=False, skip_group_check=True)
                for g in range(8):
                    nc.tensor.matmul(
                        pv[:, g * 12:(g + 1) * 12],
                        XF_cur[:, g * 1280 + t * 128:g * 1280 + (t + 1) * 128],
                        WB("A3", 12),
                        start=False, stop=False, skip_group_check=True)
                nc.tensor.matmul(
                    pv[:, 0:96], onesr[:, :128], b3rb[:, 0:96],
                    start=False, stop=True, skip_group_check=True)
                cp(ysb[:, t * 96:(t + 1) * 96], pv[:, 0:96])
            nc.sync.dma_start(ydram[:, 0:480], ysb[:, 0:480])
            nc.sync.dma_start(ydram[:, 480:960], ysb[:, 480:960])

    nc.compile()
    return nc


def kernel(**inputs):
    import sys
    for p in ("/opt/trn_rl_repo", "/opt/trn_rl_repo/concourse"):
        if p not in sys.path:
            sys.path.insert(0, p)
    from concourse.bass_utils import run_bass_kernel_spmd
    import ml_dtypes

    host = _build_host(inputs)
    woffs = host.pop("_woffs")
    xT_full = host.pop("xTp_full")

    key = ("nc",)
    if key not in _CACHE:
        _CACHE[key] = _build_nc(woffs)
    nc = _CACHE[key]

    in_maps = []
    for c in range(NCORES):
        m = dict(host)
        xc = xT_full[:, c * BL:(c + 1) * BL]  # [2048, 32]
        m["xTp"] = np.ascontiguousarray(
            xc.reshape(16, 128, BL).transpose(1, 0, 2).reshape(128, 16 * BL)
        ).astype(ml_dtypes.bfloat16)
        in_maps.append(m)
    res = run_bass_kernel_spmd(nc, in_maps, core_ids=list(range(NCORES)))
    outs = []
    for c in range(NCORES):
        y = res.results[c]["y"].astype(np.float32)  # [128, 960]
        outs.append(y.reshape(128, 10, BL, 3).transpose(2, 1, 0, 3)
                    .reshape(BL, 1280, 3))
    return np.concatenate(outs, axis=0)


if __name__ == "__main__":
    import reference as R
    inp = R.setup_inputs()
    inp = {k: np.asarray(v) for k, v in inp.items()}
    act = kernel(**inp)
    exp = np.asarray(R.reference(**inp))
    err = np.linalg.norm(act - exp) / np.linalg.norm(exp)
    print("Relative error:", err)
